# revision 17
# baseline (speedup 1.0000x reference)
# Trainium2 Bass kernel: single-head causal attention (k.q^T scores, no scale)
# B=16, T=4096, D=64. Data-parallel over batch: 2 batches per NeuronCore x 8.
# bf16 matmul datapath; software-pipelined score->exp->PV chunk stream.
# Softmax exp is split: non-diagonal chunks run native Exp on the Activation
# engine; diagonal chunks run a Schraudolph bf16 bit-trick on DVE
# (scalar_tensor_tensor) with the causal mask FUSED as an additive constant
# that drives masked lanes to int16 saturation = bf16 -0.0.
import numpy as np

B, T, D = 16, 4096, 64
NCORES = 8
BPC = B // NCORES      # batches per core
TT = 512               # t-tile width
NTT = T // TT          # 8 t tiles
SB = 128               # s block
NSB = T // SB          # 32 s blocks

# Schraudolph fast-exp constants for bf16 bit pattern:
# i16 = round(x * 128/ln2 + (127*128 - c)); bits(bf16) = i16
FEXP_MUL = 184.66496
FEXP_ADD = 16250.5
FEXP_NEG = -1.0e5      # additive mask value -> int16 saturation -> bf16 -0.0
FUSED_MASK = True      # False: diag chunks use tensor_scalar + mask multiply

_cache = {}


def _build():
    from contextlib import ExitStack
    import concourse.bass as bass
    import concourse.mybir as mybir
    import concourse.tile as tile

    f32 = mybir.dt.float32
    bf16 = mybir.dt.bfloat16
    i16 = mybir.dt.int16
    EXP = mybir.ActivationFunctionType.Exp
    MUL = mybir.AluOpType.mult
    ADD = mybir.AluOpType.add

    nc = bass.Bass("TRN2", target_bir_lowering=False, debug=False,
                   enable_asserts=False)

    xT_d = nc.dram_tensor("xtb", [BPC, D, T], bf16, kind="ExternalInput").ap()
    wq_d = nc.dram_tensor("wqb", [D, 128], bf16, kind="ExternalInput").ap()
    wk_d = nc.dram_tensor("wkb", [D, 128], bf16, kind="ExternalInput").ap()
    xr_d = nc.dram_tensor("xrb", [BPC, NSB, SB, D], bf16,
                          kind="ExternalInput").ap()
    wve_d = nc.dram_tensor("wveb", [65, 65], bf16, kind="ExternalInput").ap()
    ma_d = nc.dram_tensor("mkadd", [2, 128, 1024], f32,
                          kind="ExternalInput").ap()
    mk_d = nc.dram_tensor("mkb", [128, 512], bf16, kind="ExternalInput").ap()
    on_d = nc.dram_tensor("onesb", [128, 32], bf16, kind="ExternalInput").ap()
    out_d = nc.dram_tensor("out", [BPC, T, D], f32, kind="ExternalOutput").ap()

    with ExitStack() as ctx:
        tc = ctx.enter_context(tile.TileContext(nc))
        consts = ctx.enter_context(tc.tile_pool(name="consts", bufs=1))
        bigp = ctx.enter_context(tc.tile_pool(name="big", bufs=2))
        ptp = ctx.enter_context(tc.tile_pool(name="pt", bufs=6))
        stg = ctx.enter_context(tc.tile_pool(name="stg", bufs=4))
        # PSUM: st [128,1024]f32 x3 = 6 banks, outp [65,512]f32 x1 = 1,
        # tr [128,65]f32 x1 = 1  -> 8 banks
        pst = ctx.enter_context(tc.tile_pool(name="pst", bufs=3, space="PSUM"))
        pso = ctx.enter_context(tc.tile_pool(name="pso", bufs=1, space="PSUM"))
        ptr = ctx.enter_context(tc.tile_pool(name="ptr", bufs=1, space="PSUM"))

        wq_sb = consts.tile([D, 128], bf16, tag="wq")
        wk_sb = consts.tile([D, 128], bf16, tag="wk")
        wve_sb = consts.tile([65, 65], bf16, tag="wve")
        ma0_sb = consts.tile([128, 1024], f32, tag="ma0")
        ma1_sb = consts.tile([128, 1024], f32, tag="ma1")
        mk_sb = consts.tile([128, 512], bf16, tag="mk")
        nc.gpsimd.dma_start(wq_sb[:], wq_d[:])
        nc.gpsimd.dma_start(wk_sb[:], wk_d[:])

        # hoist both batches' input DMAs (v comes straight from x rows; Wv is
        # folded into the drain transpose) -- critical-first queue order
        xts, vexs = [], []
        for b in range(BPC):
            xt_sb = bigp.tile([D, T], bf16, tag="xt")
            vex = bigp.tile([128, NSB, 65], bf16, tag="vex")
            xts.append(xt_sb)
            vexs.append(vex)
        nc.gpsimd.dma_start(xts[0][:], xT_d[0])
        nc.gpsimd.dma_start(vexs[0][:, :, 0:64], xr_d[0].transpose([1, 0, 2]))
        nc.gpsimd.dma_start(vexs[0][:, :, 64], on_d[:])
        nc.gpsimd.dma_start(ma0_sb[:], ma_d[0])
        nc.gpsimd.dma_start(ma1_sb[:], ma_d[1])
        nc.gpsimd.dma_start(mk_sb[:], mk_d[:])
        nc.gpsimd.dma_start(wve_sb[:], wve_d[:])
        for b in range(1, BPC):
            nc.gpsimd.dma_start(xts[b][:], xT_d[b])
            nc.gpsimd.dma_start(vexs[b][:, :, 0:64],
                                xr_d[b].transpose([1, 0, 2]))
            nc.gpsimd.dma_start(vexs[b][:, :, 64], on_d[:])

        qTds, kTds = {}, {}

        def proj_thunks(b):
            def pj(i, b=b):
                if i == 0:
                    qTd = bigp.tile([128, T], bf16, tag="qtd")
                    kTd = bigp.tile([128, T], bf16, tag="ktd")
                    qTds[b], kTds[b] = qTd, kTd
                ps = pst.tile([128, 1024], f32, tag="st")
                sl = slice(i * TT, (i + 1) * TT)
                nc.tensor.matmul(ps[:, 0:512], wq_sb[:], xts[b][:, sl])
                nc.tensor.matmul(ps[:, 512:1024], wk_sb[:], xts[b][:, sl])
                nc.vector.tensor_copy(qTds[b][:, sl], ps[:, 0:512])
                nc.vector.tensor_copy(kTds[b][:, sl], ps[:, 512:1024])
            return [lambda i=i: pj(i) for i in range(NTT)]

        work = [(t, c) for t in range(NTT) for c in range(2 * (t + 1))]

        def chunk_ranges(t, c):
            """exp ranges [(col0, col1)] in the [128,1024] st tile, one
            per 512-half so PV p0 never waits on the p1 half's exp."""
            if c == 2 * t + 1:
                return [(256, 512), (896, 1024)]
            return [(0, 512), (512, 1024)]

        next_proj = proj_thunks(0)
        next_proj.pop(0)()  # batch 0 proj tile 0 up front

        for b in range(BPC):
            vex = vexs[b]
            own_proj = next_proj
            next_proj = proj_thunks(b + 1) if b + 1 < BPC else []
            qTd, kTd = qTds[b], kTds[b]

            def scores(t, c):
                st = pst.tile([128, 1024], f32, tag="st")
                for p in range(2):
                    sblk = 2 * c + p
                    j = sblk - 4 * t
                    lo = 128 * j if (c == 2 * t + 1 and j > 0) else 0
                    half = slice(64 * p, 64 * (p + 1))
                    nc.tensor.matmul(
                        st[:, 512 * p + lo: 512 * (p + 1)],
                        qTd[half, sblk * SB:(sblk + 1) * SB],
                        kTd[half, t * TT + lo:(t + 1) * TT])
                return st

            def do_exp(t, c, st):
                pt = ptp.tile([128, 1024], bf16, tag="pt")
                diag = c >= 2 * t
                # a slice of inner chunks also runs fast-exp on DVE to keep
                # the Act engine below the PE's per-chunk budget
                on_dve = (not diag) and (c % 8 == 2)
                for (a, z) in chunk_ranges(t, c):
                    if diag and FUSED_MASK:
                        ma = ma1_sb if c == 2 * t + 1 else ma0_sb
                        nc.vector.scalar_tensor_tensor(
                            pt[:, a:z].bitcast(i16), st[:, a:z],
                            FEXP_MUL, ma[:, a:z], MUL, ADD)
                    elif diag or on_dve:
                        nc.vector.tensor_scalar(
                            pt[:, a:z].bitcast(i16), st[:, a:z],
                            FEXP_MUL, FEXP_ADD, MUL, ADD)
                    else:
                        nc.scalar.activation(pt[:, a:z], st[:, a:z], EXP)
                return pt

            def mask_pv(t, c, pt, outp):
                for p in range(2):
                    sblk = 2 * c + p
                    j = sblk - 4 * t
                    lo = 128 * j if j >= 0 else 0
                    if j >= 0 and not FUSED_MASK:
                        nc.vector.tensor_mul(
                            pt[:, 512 * p + lo: 512 * (p + 1)],
                            pt[:, 512 * p + lo: 512 * (p + 1)],
                            mk_sb[:, 0: TT - lo])
                    nc.tensor.matmul(
                        outp[:, lo:TT],
                        vex[:, sblk, :],
                        pt[:, 512 * p + lo: 512 * (p + 1)],
                        start=(sblk == 0), stop=(sblk == 4 * t + 3))

            drain_steps = []

            def start_drain(t, outp, b):
                # PSUM->SBUF copy right away; Wv application + normalize
                # spread one sub-step per chunk
                ot = stg.tile([65, TT], bf16, tag="ot")
                nc.vector.tensor_copy(ot[:], outp[:])
                trs = [None] * 4

                def tstep(i, ot=ot):
                    tr = ptr.tile([128, 65], f32, tag="tr")
                    trs[i] = tr
                    nc.tensor.matmul(tr[:], ot[:, 128 * i: 128 * (i + 1)],
                                     wve_sb[:])

                def nstep(i, t=t, b=b):
                    tr = trs[i]
                    rcp = stg.tile([128, 1], f32, tag="rcp")
                    nc.vector.reciprocal(rcp[:], tr[:, 64:65])
                    on = stg.tile([128, 64], f32, tag="on")
                    nc.vector.tensor_scalar_mul(on[:], tr[:, 0:64], rcp[:])
                    r0 = t * TT + 128 * i
                    nc.sync.dma_start(out_d[b, r0:r0 + 128, :], on[:])

                drain_steps.append(None)  # 1-chunk delay
                for k in range(4):
                    drain_steps.append(lambda k=k: tstep(k))
                    drain_steps.append(lambda k=k: nstep(k))

            # two-chunk score lookahead so the PE never drains while a
            # cross-engine exp sits on the critical path
            st_by_i = {0: scores(*work[0]), 1: scores(*work[1])}
            outp = None
            nwork = len(work)
            for i, (t, c) in enumerate(work):
                pt = do_exp(t, c, st_by_i.pop(i))
                if i + 2 < nwork:
                    st_by_i[i + 2] = scores(*work[i + 2])
                if drain_steps:
                    s0 = drain_steps.pop(0)
                    if s0 is not None:
                        s0()
                if c == 0:
                    outp = pso.tile([65, TT], f32, tag="o")
                mask_pv(t, c, pt, outp)
                # own projection: one tile per early chunk (just-in-time);
                # next batch's projection: every other chunk of the tail
                if own_proj:
                    own_proj.pop(0)()
                elif next_proj and i >= nwork - 2 * len(
                        proj_thunks(0)) and (nwork - i) % 2 == 0:
                    next_proj.pop(0)()
                if c == 2 * (t + 1) - 1:
                    start_drain(t, outp, b)
            while drain_steps:
                s0 = drain_steps.pop(0)
                if s0 is not None:
                    s0()
            while next_proj and b + 1 < BPC:
                next_proj.pop(0)()

    _split_matmul_waits(nc)
    return nc


def _split_matmul_waits(nc):
    """Matmults lower via an LDWEIGHTS struct with a single ISA wait slot;
    walrus refuses Matmult instructions carrying >1 sync wait. Move every
    multi-wait Matmult's waits onto a PE NoOp inserted right before it
    (engines execute their stream in order, so this is equivalent)."""
    import bass_rust
    import concourse.mybir as mybir
    moved = 0
    for fn in nc.m.functions:
        for bb in fn.blocks:
            il = bb.instructions
            k = 0
            while k < len(il):
                inst = il[k]
                if inst.opcode != "NoOp":
                    si = inst.sync_info
                    if si is not None and si.on_wait and len(si.on_wait) > 1:
                        waits = list(si.on_wait)
                        ups = list(si.on_update) if si.on_update else []
                        for wi, w in enumerate(waits):
                            nop = mybir.InstNoOp(name=f"{inst.name}-ws{wi}",
                                                 ins=[], outs=[])
                            nop.engine = inst.engine
                            nop.sync_info = bass_rust.SyncInfo(
                                on_wait=[w], on_update=[])
                            il.insert(k, nop)
                            k += 1
                        inst.sync_info = bass_rust.SyncInfo(
                            on_wait=[], on_update=ups)
                        moved += 1
                k += 1
    return moved


def _get_nc():
    if "nc" not in _cache:
        _cache["nc"] = _build()
    return _cache["nc"]


def _mask_adds():
    """Additive fast-exp mask constants [2,128,1024] fp32: FEXP_ADD on valid
    lanes, FEXP_NEG on masked lanes (int16 saturation -> bf16 -0.0).
    Plane 0: chunk c==2t (j=0 at cols 0:128, j=1 at 640:768 after the
    dead 512:640 region). Plane 1: chunk c==2t+1 (j=2 at 256:384, j=3 at
    896:1024)."""
    s = np.arange(128)[:, None]
    col = np.arange(128)[None, :]
    tri_masked = col < s  # [s, col] masked above the diagonal
    m0 = np.full((128, 1024), FEXP_ADD, np.float32)
    m0[:, 0:128][tri_masked] = FEXP_NEG
    m0[:, 512:640] = FEXP_NEG
    m0[:, 640:768][tri_masked] = FEXP_NEG
    m1 = np.full((128, 1024), FEXP_ADD, np.float32)
    m1[:, 0:256] = FEXP_NEG
    m1[:, 256:384][tri_masked] = FEXP_NEG
    m1[:, 512:896] = FEXP_NEG
    m1[:, 896:1024][tri_masked] = FEXP_NEG
    return np.ascontiguousarray(np.stack([m0, m1]))


def kernel(x, Wk, Wq, Wv):
    from concourse.bass_utils import run_bass_kernel_spmd
    import ml_dtypes

    bf = ml_dtypes.bfloat16
    x = np.asarray(x, dtype=np.float32)
    Wk = np.asarray(Wk, dtype=np.float32)
    Wq = np.asarray(Wq, dtype=np.float32)
    Wv = np.asarray(Wv, dtype=np.float32)

    xT = np.ascontiguousarray(x.transpose(0, 2, 1).astype(bf))  # [B, D, T]
    wq2 = np.ascontiguousarray(
        np.concatenate([Wq.T, Wq.T], axis=1).astype(bf))        # [64, 128]
    wk2 = np.ascontiguousarray(
        np.concatenate([Wk.T, Wk.T], axis=1).astype(bf))
    xr = np.ascontiguousarray(x.astype(bf)).reshape(B, NSB, SB, D)
    wve = np.zeros((65, 65), dtype=bf)
    wve[0:64, 0:64] = Wv.T.astype(bf)
    wve[64, 64] = 1.0
    maskw = np.ones((128, 512), dtype=bf)
    maskw[:, 0:128] = np.triu(np.ones((128, 128), dtype=np.float32)).astype(bf)

    nc = _get_nc()
    in_maps = []
    for c in range(NCORES):
        in_maps.append({
            "xtb": np.ascontiguousarray(xT[BPC * c: BPC * (c + 1)]),
            "wqb": wq2, "wkb": wk2,
            "xrb": np.ascontiguousarray(xr[BPC * c: BPC * (c + 1)]),
            "wveb": wve,
            "mkadd": _mask_adds(),
            "mkb": maskw,
            "onesb": np.ones((128, 32), dtype=bf),
        })
    import os
    kw = {}
    if os.environ.get("BASS_TRACE"):
        kw = dict(trace=True, stitch_traces=False)
    res = run_bass_kernel_spmd(nc, in_maps, core_ids=list(range(NCORES)), **kw)
    _cache["last_result"] = res
    out = np.empty((B, T, D), dtype=np.float32)
    for c in range(NCORES):
        out[BPC * c: BPC * (c + 1)] = res.results[c]["out"]
    return out


# revision 18
# speedup vs baseline: 1.0402x; 1.0402x over previous
# Trainium2 Bass kernel: single-head causal attention (k.q^T scores, no scale)
# B=16, T=4096, D=64. Data-parallel over batch: 2 batches per NeuronCore x 8.
# bf16 matmul datapath; software-pipelined score->exp->PV chunk stream.
# Softmax exp is split: non-diagonal chunks run native Exp on the Activation
# engine; diagonal chunks run a Schraudolph bf16 bit-trick on DVE
# (scalar_tensor_tensor) with the causal mask FUSED as an additive constant
# that drives masked lanes to int16 saturation = bf16 -0.0.
import numpy as np

B, T, D = 16, 4096, 64
NCORES = 8
BPC = B // NCORES      # batches per core
TT = 512               # t-tile width
NTT = T // TT          # 8 t tiles
SB = 128               # s block
NSB = T // SB          # 32 s blocks

# Schraudolph fast-exp constants for bf16 bit pattern:
# i16 = round(x * 128/ln2 + (127*128 - c)); bits(bf16) = i16
FEXP_MUL = 184.66496
FEXP_ADD = 16250.5
FEXP_NEG = -1.0e5      # additive mask value -> int16 saturation -> bf16 -0.0
FUSED_MASK = True      # False: diag chunks use tensor_scalar + mask multiply

_cache = {}


def _build():
    from contextlib import ExitStack
    import concourse.bass as bass
    import concourse.mybir as mybir
    import concourse.tile as tile

    f32 = mybir.dt.float32
    bf16 = mybir.dt.bfloat16
    i16 = mybir.dt.int16
    EXP = mybir.ActivationFunctionType.Exp
    MUL = mybir.AluOpType.mult
    ADD = mybir.AluOpType.add

    nc = bass.Bass("TRN2", target_bir_lowering=False, debug=False,
                   enable_asserts=False)

    xT_d = nc.dram_tensor("xtb", [BPC, D, T], bf16, kind="ExternalInput").ap()
    wq_d = nc.dram_tensor("wqb", [D, 128], bf16, kind="ExternalInput").ap()
    wk_d = nc.dram_tensor("wkb", [D, 128], bf16, kind="ExternalInput").ap()
    xr_d = nc.dram_tensor("xrb", [BPC, NSB, SB, D], bf16,
                          kind="ExternalInput").ap()
    wve_d = nc.dram_tensor("wveb", [65, 65], bf16, kind="ExternalInput").ap()
    ma_d = nc.dram_tensor("mkadd", [2, 128, 1024], f32,
                          kind="ExternalInput").ap()
    mk_d = nc.dram_tensor("mkb", [128, 512], bf16, kind="ExternalInput").ap()
    on_d = nc.dram_tensor("onesb", [128, 32], bf16, kind="ExternalInput").ap()
    out_d = nc.dram_tensor("out", [BPC, T, D], f32, kind="ExternalOutput").ap()

    with ExitStack() as ctx:
        tc = ctx.enter_context(tile.TileContext(nc))
        consts = ctx.enter_context(tc.tile_pool(name="consts", bufs=1))
        bigp = ctx.enter_context(tc.tile_pool(name="big", bufs=2))
        ptp = ctx.enter_context(tc.tile_pool(name="pt", bufs=6))
        stg = ctx.enter_context(tc.tile_pool(name="stg", bufs=4))
        # PSUM: st [128,1024]f32 x3 = 6 banks, outp [65,512]f32 x1 = 1,
        # tr [128,65]f32 x1 = 1  -> 8 banks
        pst = ctx.enter_context(tc.tile_pool(name="pst", bufs=3, space="PSUM"))
        pso = ctx.enter_context(tc.tile_pool(name="pso", bufs=1, space="PSUM"))
        ptr = ctx.enter_context(tc.tile_pool(name="ptr", bufs=1, space="PSUM"))

        wq_sb = consts.tile([D, 128], bf16, tag="wq")
        wk_sb = consts.tile([D, 128], bf16, tag="wk")
        wve_sb = consts.tile([65, 65], bf16, tag="wve")
        ma0_sb = consts.tile([128, 1024], f32, tag="ma0")
        ma1_sb = consts.tile([128, 1024], f32, tag="ma1")
        mk_sb = consts.tile([128, 512], bf16, tag="mk")
        nc.gpsimd.dma_start(wq_sb[:], wq_d[:])
        nc.gpsimd.dma_start(wk_sb[:], wk_d[:])

        # hoist both batches' input DMAs (v comes straight from x rows; Wv is
        # folded into the drain transpose) -- critical-first queue order
        xts, vexs = [], []
        for b in range(BPC):
            xt_sb = bigp.tile([D, T], bf16, tag="xt")
            vex = bigp.tile([128, NSB, 65], bf16, tag="vex")
            xts.append(xt_sb)
            vexs.append(vex)
        nc.gpsimd.dma_start(xts[0][:], xT_d[0])
        nc.gpsimd.dma_start(vexs[0][:, :, 0:64], xr_d[0].transpose([1, 0, 2]))
        nc.gpsimd.dma_start(vexs[0][:, :, 64], on_d[:])
        nc.gpsimd.dma_start(ma0_sb[:], ma_d[0])
        nc.gpsimd.dma_start(ma1_sb[:], ma_d[1])
        nc.gpsimd.dma_start(mk_sb[:], mk_d[:])
        nc.gpsimd.dma_start(wve_sb[:], wve_d[:])
        for b in range(1, BPC):
            nc.gpsimd.dma_start(xts[b][:], xT_d[b])
            nc.gpsimd.dma_start(vexs[b][:, :, 0:64],
                                xr_d[b].transpose([1, 0, 2]))
            nc.gpsimd.dma_start(vexs[b][:, :, 64], on_d[:])

        qTds, kTds = {}, {}

        def proj_thunks(b):
            def pj(i, b=b):
                if i == 0:
                    qTd = bigp.tile([128, T], bf16, tag="qtd")
                    kTd = bigp.tile([128, T], bf16, tag="ktd")
                    qTds[b], kTds[b] = qTd, kTd
                ps = pst.tile([128, 1024], f32, tag="st")
                sl = slice(i * TT, (i + 1) * TT)
                nc.tensor.matmul(ps[:, 0:512], wq_sb[:], xts[b][:, sl])
                nc.tensor.matmul(ps[:, 512:1024], wk_sb[:], xts[b][:, sl])
                nc.vector.tensor_copy(qTds[b][:, sl], ps[:, 0:512])
                nc.vector.tensor_copy(kTds[b][:, sl], ps[:, 512:1024])
            return [lambda i=i: pj(i) for i in range(NTT)]

        work = [(t, c) for t in range(NTT) for c in range(2 * (t + 1))]

        def chunk_ranges(t, c):
            """exp ranges [(col0, col1)] in the [128,1024] st tile."""
            if c == 2 * t + 1:
                return [(256, 512), (896, 1024)]
            return [(0, 1024)]

        next_proj = proj_thunks(0)
        next_proj.pop(0)()  # batch 0 proj tile 0 up front

        for b in range(BPC):
            vex = vexs[b]
            own_proj = next_proj
            next_proj = proj_thunks(b + 1) if b + 1 < BPC else []
            qTd, kTd = qTds[b], kTds[b]

            def scores(t, c):
                st = pst.tile([128, 1024], f32, tag="st")
                for p in range(2):
                    sblk = 2 * c + p
                    j = sblk - 4 * t
                    lo = 128 * j if (c == 2 * t + 1 and j > 0) else 0
                    half = slice(64 * p, 64 * (p + 1))
                    nc.tensor.matmul(
                        st[:, 512 * p + lo: 512 * (p + 1)],
                        qTd[half, sblk * SB:(sblk + 1) * SB],
                        kTd[half, t * TT + lo:(t + 1) * TT])
                return st

            def do_exp(t, c, st):
                """Returns (pt, fused). Early tiles (t<4) are diag-dense and
                drain-dense: their diag exps go to the then-idle Act engine
                (mask applied separately). Late tiles: diag fused on DVE, and
                25% of inner chunks offloaded to DVE fast-exp to keep Act
                under the PE per-chunk budget."""
                pt = ptp.tile([128, 1024], bf16, tag="pt")
                diag = c >= 2 * t
                fused = diag and FUSED_MASK and t >= 4
                on_dve = (not diag) and (c % 4 == 2)
                for (a, z) in chunk_ranges(t, c):
                    if fused:
                        ma = ma1_sb if c == 2 * t + 1 else ma0_sb
                        nc.vector.scalar_tensor_tensor(
                            pt[:, a:z].bitcast(i16), st[:, a:z],
                            FEXP_MUL, ma[:, a:z], MUL, ADD)
                    elif on_dve:
                        nc.vector.tensor_scalar(
                            pt[:, a:z].bitcast(i16), st[:, a:z],
                            FEXP_MUL, FEXP_ADD, MUL, ADD)
                    else:
                        nc.scalar.activation(pt[:, a:z], st[:, a:z], EXP)
                return pt, fused

            def mask_pv(t, c, pt, outp, fused):
                for p in range(2):
                    sblk = 2 * c + p
                    j = sblk - 4 * t
                    lo = 128 * j if j >= 0 else 0
                    if j >= 0 and not fused:
                        nc.vector.tensor_mul(
                            pt[:, 512 * p + lo: 512 * (p + 1)],
                            pt[:, 512 * p + lo: 512 * (p + 1)],
                            mk_sb[:, 0: TT - lo])
                    nc.tensor.matmul(
                        outp[:, lo:TT],
                        vex[:, sblk, :],
                        pt[:, 512 * p + lo: 512 * (p + 1)],
                        start=(sblk == 0), stop=(sblk == 4 * t + 3))

            drain_steps = []

            def start_drain(t, outp, b):
                # PSUM->SBUF copy right away; Wv application + normalize
                # spread one sub-step per chunk
                ot = stg.tile([65, TT], bf16, tag="ot")
                if t < 4:
                    nc.scalar.copy(ot[:], outp[:])
                else:
                    nc.vector.tensor_copy(ot[:], outp[:])
                trs = [None] * 4

                def tstep(i, ot=ot):
                    tr = ptr.tile([128, 65], f32, tag="tr")
                    trs[i] = tr
                    nc.tensor.matmul(tr[:], ot[:, 128 * i: 128 * (i + 1)],
                                     wve_sb[:])

                def nstep(i, t=t, b=b):
                    tr = trs[i]
                    rcp = stg.tile([128, 1], f32, tag="rcp")
                    nc.vector.reciprocal(rcp[:], tr[:, 64:65])
                    on = stg.tile([128, 64], f32, tag="on")
                    nc.vector.tensor_scalar_mul(on[:], tr[:, 0:64], rcp[:])
                    r0 = t * TT + 128 * i
                    nc.sync.dma_start(out_d[b, r0:r0 + 128, :], on[:])

                drain_steps.append(None)  # 1-chunk delay
                for k in range(4):
                    drain_steps.append(lambda k=k: tstep(k))
                    drain_steps.append(lambda k=k: nstep(k))

            # two-chunk score lookahead so the PE never drains while a
            # cross-engine exp sits on the critical path
            st_by_i = {0: scores(*work[0]), 1: scores(*work[1])}
            outp = None
            nwork = len(work)
            for i, (t, c) in enumerate(work):
                pt, fused = do_exp(t, c, st_by_i.pop(i))
                if i + 2 < nwork:
                    st_by_i[i + 2] = scores(*work[i + 2])
                if drain_steps:
                    s0 = drain_steps.pop(0)
                    if s0 is not None:
                        s0()
                if c == 0:
                    outp = pso.tile([65, TT], f32, tag="o")
                mask_pv(t, c, pt, outp, fused)
                # own projection: one tile per early chunk (just-in-time);
                # next batch's projection: every other chunk of the tail
                if own_proj:
                    own_proj.pop(0)()
                elif next_proj and i >= nwork - 2 * len(
                        proj_thunks(0)) and (nwork - i) % 2 == 0:
                    next_proj.pop(0)()
                if c == 2 * (t + 1) - 1:
                    start_drain(t, outp, b)
            while drain_steps:
                s0 = drain_steps.pop(0)
                if s0 is not None:
                    s0()
            while next_proj and b + 1 < BPC:
                next_proj.pop(0)()

    _split_matmul_waits(nc)
    return nc


def _split_matmul_waits(nc):
    """Matmults lower via an LDWEIGHTS struct with a single ISA wait slot;
    walrus refuses Matmult instructions carrying >1 sync wait. Move every
    multi-wait Matmult's waits onto a PE NoOp inserted right before it
    (engines execute their stream in order, so this is equivalent)."""
    import bass_rust
    import concourse.mybir as mybir
    moved = 0
    for fn in nc.m.functions:
        for bb in fn.blocks:
            il = bb.instructions
            k = 0
            while k < len(il):
                inst = il[k]
                if inst.opcode != "NoOp":
                    si = inst.sync_info
                    if si is not None and si.on_wait and len(si.on_wait) > 1:
                        waits = list(si.on_wait)
                        ups = list(si.on_update) if si.on_update else []
                        for wi, w in enumerate(waits):
                            nop = mybir.InstNoOp(name=f"{inst.name}-ws{wi}",
                                                 ins=[], outs=[])
                            nop.engine = inst.engine
                            nop.sync_info = bass_rust.SyncInfo(
                                on_wait=[w], on_update=[])
                            il.insert(k, nop)
                            k += 1
                        inst.sync_info = bass_rust.SyncInfo(
                            on_wait=[], on_update=ups)
                        moved += 1
                k += 1
    return moved


def _get_nc():
    if "nc" not in _cache:
        _cache["nc"] = _build()
    return _cache["nc"]


def _mask_adds():
    """Additive fast-exp mask constants [2,128,1024] fp32: FEXP_ADD on valid
    lanes, FEXP_NEG on masked lanes (int16 saturation -> bf16 -0.0).
    Plane 0: chunk c==2t (j=0 at cols 0:128, j=1 at 640:768 after the
    dead 512:640 region). Plane 1: chunk c==2t+1 (j=2 at 256:384, j=3 at
    896:1024)."""
    s = np.arange(128)[:, None]
    col = np.arange(128)[None, :]
    tri_masked = col < s  # [s, col] masked above the diagonal
    m0 = np.full((128, 1024), FEXP_ADD, np.float32)
    m0[:, 0:128][tri_masked] = FEXP_NEG
    m0[:, 512:640] = FEXP_NEG
    m0[:, 640:768][tri_masked] = FEXP_NEG
    m1 = np.full((128, 1024), FEXP_ADD, np.float32)
    m1[:, 0:256] = FEXP_NEG
    m1[:, 256:384][tri_masked] = FEXP_NEG
    m1[:, 512:896] = FEXP_NEG
    m1[:, 896:1024][tri_masked] = FEXP_NEG
    return np.ascontiguousarray(np.stack([m0, m1]))


def kernel(x, Wk, Wq, Wv):
    from concourse.bass_utils import run_bass_kernel_spmd
    import ml_dtypes

    bf = ml_dtypes.bfloat16
    x = np.asarray(x, dtype=np.float32)
    Wk = np.asarray(Wk, dtype=np.float32)
    Wq = np.asarray(Wq, dtype=np.float32)
    Wv = np.asarray(Wv, dtype=np.float32)

    xT = np.ascontiguousarray(x.transpose(0, 2, 1).astype(bf))  # [B, D, T]
    wq2 = np.ascontiguousarray(
        np.concatenate([Wq.T, Wq.T], axis=1).astype(bf))        # [64, 128]
    wk2 = np.ascontiguousarray(
        np.concatenate([Wk.T, Wk.T], axis=1).astype(bf))
    xr = np.ascontiguousarray(x.astype(bf)).reshape(B, NSB, SB, D)
    wve = np.zeros((65, 65), dtype=bf)
    wve[0:64, 0:64] = Wv.T.astype(bf)
    wve[64, 64] = 1.0
    maskw = np.ones((128, 512), dtype=bf)
    maskw[:, 0:128] = np.triu(np.ones((128, 128), dtype=np.float32)).astype(bf)

    nc = _get_nc()
    in_maps = []
    for c in range(NCORES):
        in_maps.append({
            "xtb": np.ascontiguousarray(xT[BPC * c: BPC * (c + 1)]),
            "wqb": wq2, "wkb": wk2,
            "xrb": np.ascontiguousarray(xr[BPC * c: BPC * (c + 1)]),
            "wveb": wve,
            "mkadd": _mask_adds(),
            "mkb": maskw,
            "onesb": np.ones((128, 32), dtype=bf),
        })
    import os
    kw = {}
    if os.environ.get("BASS_TRACE"):
        kw = dict(trace=True, stitch_traces=False)
    res = run_bass_kernel_spmd(nc, in_maps, core_ids=list(range(NCORES)), **kw)
    _cache["last_result"] = res
    out = np.empty((B, T, D), dtype=np.float32)
    for c in range(NCORES):
        out[BPC * c: BPC * (c + 1)] = res.results[c]["out"]
    return out
